# revision 9
# baseline (speedup 1.0000x reference)
"""Trainium2 Bass kernel for the CMS/HOPE memory cell.

Sharding: data-parallel over batch B=8 across 8 NeuronCores (one batch
element per core, no collectives). Inside each core:
  - q-network (small MLP) on TensorE
  - per level l: stream K_l once (scores on VectorE, decay-scale on ScalarE,
    write K_l*keep out), softmax stats, stream M_l once (context via PSUM
    matmul accumulation, decay-scale on VectorE, write M_l*keep out)
  - mixing + HOPE core + CMSWrite row-0 patches on TensorE/VectorE/ScalarE.
All shapes are hardcoded; kernel() takes full unsharded inputs.
"""

import sys

sys.path.insert(0, "/opt/trn_rl_repo")

import math
from contextlib import ExitStack

import numpy as np

import concourse.bass as bass
import concourse.tile as tile
from concourse import bacc
from concourse import bass_isa, mybir
from concourse.masks import make_identity

F32 = mybir.dt.float32
AF = mybir.ActivationFunctionType
AX = mybir.AxisListType

B = 8
D_S, D_E, D_K, D_C, D_P, D_W = 512, 512, 128, 256, 256, 256
SIZES = [32768, 8192, 2048]
DIMS = [256, 256, 256]
DECAYS = [0.001, 0.01, 0.1]
NLVL = 3
SCALE = 1.0 / math.sqrt(float(D_K))
SUB = 128          # rows per sub-tile (one partition block)
SUP_K = 16         # sub-tiles per K super-tile (2048 rows, 1 MiB)
SUP_M = 8          # sub-tiles per M super-tile (1024 rows, 1 MiB)
H1 = D_S + D_E     # 1024
H2 = D_P + D_C     # 512


def build_nc():
    nc = bacc.Bacc()

    # ---------------- I/O declarations (per-core slices) ----------------
    h = {}

    def din(name, shape):
        h[name] = nc.declare_dram_parameter(name, list(shape), F32, isOutput=False)
        return h[name]

    def dout(name, shape):
        h[name] = nc.declare_dram_parameter(name, list(shape), F32, isOutput=True)
        return h[name]

    for l in range(NLVL):
        din(f"M{l}", (SIZES[l], DIMS[l]))
        din(f"K{l}", (SIZES[l], D_K))
    din("s_prev", (1, D_S))
    din("w_prev", (1, D_W))
    din("p_prev", (1, D_P))
    din("e_t", (1, D_E))
    din("r_t", (1, 1))
    # params
    din("Wq1", (H1, H1)); din("bq1", (1, H1)); din("gq", (1, H1)); din("betq", (1, H1))
    din("Wq2", (H1, D_K)); din("bq2", (1, D_K))
    for l in range(NLVL):
        din(f"Wproj{l}", (DIMS[l], D_C))
    din("Wmix", (NLVL * D_C, NLVL)); din("bmix", (1, NLVL))
    din("Wz", (D_E + D_C, D_P)); din("bz", (1, D_P))
    din("Wg", (D_S + D_E + D_C, D_S)); din("bg", (1, D_S))
    din("U_p", (D_S, D_P)); din("U_w", (D_S, D_W))
    din("A_diag", (1, D_W)); din("A_U", (D_W, D_W // 4)); din("A_V", (D_W, D_W // 4))
    din("Wmod", (D_C, D_W)); din("bmod", (1, D_W))
    din("WB", (D_C + D_P, D_W)); din("bB", (1, D_W))
    din("Wp1", (D_P + D_C, H2)); din("bp1", (1, H2)); din("gp", (1, H2)); din("betp", (1, H2))
    din("Wp2", (H2, D_P)); din("bp2", (1, D_P))
    for l in range(NLVL):
        din(f"Wwg{l}", (D_S + D_E + 1, 1)); din(f"bwg{l}", (1, 1))
        din(f"Wwv{l}", (D_S + D_E, DIMS[l])); din(f"bwv{l}", (1, DIMS[l]))
        din(f"Wwk{l}", (D_S + D_E, D_K)); din(f"bwk{l}", (1, D_K))

    dout("s_t", (1, D_S))
    for l in range(NLVL):
        dout(f"Mo{l}", (SIZES[l], DIMS[l]))
        dout(f"Ko{l}", (SIZES[l], D_K))

    with tile.TileContext(nc) as tc, ExitStack() as ctx:
        spool = ctx.enter_context(tc.tile_pool(name="spool", bufs=1))
        wchunk = ctx.enter_context(tc.tile_pool(name="wchunk", bufs=2))
        drpool = ctx.enter_context(tc.tile_pool(name="drpool", bufs=1, space="DRAM"))
        mpool = ctx.enter_context(tc.tile_pool(name="mpool", bufs=2))
        kpool = ctx.enter_context(tc.tile_pool(name="kpool", bufs=2))
        prodp = ctx.enter_context(tc.tile_pool(name="prodp", bufs=1))
        pctx = ctx.enter_context(tc.tile_pool(name="pctx", bufs=2, space="PSUM"))
        psmall = ctx.enter_context(tc.tile_pool(name="psmall", bufs=2, space="PSUM"))
        ptr = ctx.enter_context(tc.tile_pool(name="ptr", bufs=2, space="PSUM"))

        def pt(pool, shape, tag):
            return pool.tile(list(shape), F32, name=tag, tag=tag)

        def st(shape, tag):
            return pt(spool, shape, tag)

        def load_row(name, D, pool=None):
            t = pt(pool or spool, (1, D), "r_" + name)
            nc.sync.dma_start(out=t, in_=h[name][:])
            return t

        ident = st((128, 128), "ident")
        make_identity(nc, ident)
        eps_t = st((1, 1), "eps_t")
        nc.vector.memset(eps_t, 1e-6)

        rows = {}
        for name, D in [
            ("s_prev", D_S), ("w_prev", D_W), ("p_prev", D_P), ("e_t", D_E),
            ("r_t", 1), ("bq2", D_K),
            ("bmix", NLVL), ("bz", D_P), ("bg", D_S), ("A_diag", D_W),
            ("bmod", D_W), ("bB", D_W), ("bp1", H2), ("gp", H2), ("betp", H2),
            ("bp2", D_P),
        ]:
            rows[name] = load_row(name, D)
        for l in range(NLVL):
            for name, D in [(f"bwg{l}", 1), (f"bwv{l}", DIMS[l]), (f"bwk{l}", D_K)]:
                rows[name] = load_row(name, D)

        # ---- helpers ----
        def cols_from_dram(ap_1d, D, tag, dest=None, dcol0=0):
            """DRAM [D] -> SBUF cols tile [p, D/p] with cols[p, c] = x[c*p + p]."""
            p = min(D, 128)
            k = D // p
            t = dest if dest is not None else st((p, k), tag)
            src = ap_1d.rearrange("(c p) -> p c", p=p)
            if dest is None:
                nc.sync.dma_start(out=t, in_=src)
                return t
            nc.sync.dma_start(out=t[:, dcol0:dcol0 + k], in_=src)
            return t

        bounce_i = [0]

        def bounce(row_ap, D, tag=None):
            """SBUF row [1, D] -> SBUF cols [p, D/p] via a DRAM round-trip."""
            i = bounce_i[0]
            bounce_i[0] += 1
            tag = tag or f"bnc{i}"
            dr = pt(drpool, (1, D), tag + "_d")
            nc.sync.dma_start(out=dr, in_=row_ap)
            return cols_from_dram(dr[0, :], D, tag + "_c")

        def load_w(name, Din, Dout, pool, tag=None):
            """W [Din, Dout] -> SBUF [128, Din/128, Dout] (rhs layout)."""
            k = Din // 128
            t = pt(pool, (128, k, Dout), tag or ("w_" + name))
            nc.sync.dma_start(out=t, in_=h[name][:].rearrange("(c p) o -> p c o", p=128))
            return t

        def mv(xcols, Wsb, Dout, out_row, bias_row=None, act=None, nK=None):
            """out_row[1, Dout] = act(xcols-vector @ W + bias)."""
            k = nK if nK is not None else Wsb.shape[1]
            for f0 in range(0, Dout, 512):
                f1 = min(Dout, f0 + 512)
                ps = pt(psmall, (1, f1 - f0), "psmv")
                for c in range(k):
                    nc.tensor.matmul(ps, lhsT=xcols[:, c:c + 1], rhs=Wsb[:, c, f0:f1],
                                     start=(c == 0), stop=(c == k - 1))
                if bias_row is not None:
                    nc.vector.tensor_add(out_row[:, f0:f1], ps, bias_row[:, f0:f1])
                else:
                    nc.vector.tensor_copy(out_row[:, f0:f1], ps)
            if act is not None:
                nc.scalar.activation(out_row, out_row, act)

        def load_w_T(name, Din, Dout, pool, tag):
            """W [Din, Dout] -> transposed SBUF tile [pc, Dout/pc, Din] so that
            WT[p, jc, i] = W[i, jc*pc + p] (rhs layout for x @ W.T)."""
            pc = min(Dout, 128)
            cc = Dout // pc
            rc = Din // 128
            WT = pt(pool, (pc, cc, Din), tag)
            for r in range(rc):
                chk = pt(wchunk, (128, Dout), "wtchunk")
                nc.sync.dma_start(out=chk, in_=h[name][:][r * 128:(r + 1) * 128, :])
                for j in range(cc):
                    p_ = pt(ptr, (pc, 128), "ptrt")
                    nc.tensor.transpose(p_, chk[:, j * pc:(j + 1) * pc], ident)
                    nc.vector.tensor_copy(WT[:, j, r * 128:(r + 1) * 128], p_)
            return WT

        def layer_norm_relu(rw, Dn, g_row, b_row, tag):
            nsub = (Dn + 511) // 512
            stats = st((1, nsub, 6), tag + "_st")
            for i in range(nsub):
                nc.vector.bn_stats(stats[:, i, :], rw[:, i * 512:(i + 1) * 512])
            mvv = st((1, 2), tag + "_mv")
            nc.vector.bn_aggr(mvv, stats)
            sd = st((1, 1), tag + "_sd")
            nc.scalar.activation(sd, mvv[:, 1:2], AF.Sqrt, bias=eps_t)
            rs = st((1, 1), tag + "_rs")
            nc.vector.reciprocal(rs, sd)
            nc.vector.tensor_scalar_sub(rw, rw, mvv[:, 0:1])
            nc.vector.tensor_scalar_mul(rw, rw, rs)
            nc.vector.tensor_mul(rw, rw, g_row)
            nc.vector.tensor_add(rw, rw, b_row)
            nc.scalar.activation(rw, rw, AF.Relu)

        def bcast_mid(ap2d, n):
            return bass.AP(tensor=ap2d.tensor, offset=ap2d.offset,
                           ap=[ap2d.ap[0], [0, n], ap2d.ap[1]])

        # ================= q network (weights in a released pool) ==========
        x0c = st((128, 8), "x0c")
        cols_from_dram(h["s_prev"][0, :], D_S, None, dest=x0c, dcol0=0)
        cols_from_dram(h["e_t"][0, :], D_E, None, dest=x0c, dcol0=4)
        qb = st((128, D_K), "qb")
        qrow = st((1, D_K), "qrow")

        with tc.tile_pool(name="wqp", bufs=1) as wqp:
            wq1 = load_w("Wq1", H1, H1, wqp)
            wq2 = load_w("Wq2", H1, D_K, wqp)
            bq1r = load_row("bq1", H1, wqp)
            gqr = load_row("gq", H1, wqp)
            betqr = load_row("betq", H1, wqp)
            q1row = pt(wqp, (1, H1), "q1row")
            mv(x0c, wq1, H1, q1row, bias_row=bq1r)
            layer_norm_relu(q1row, H1, gqr, betqr, "lnq")
            qcols = bounce(q1row, H1, "qc")
            mv(qcols, wq2, D_K, qrow, bias_row=rows["bq2"])
            nc.gpsimd.partition_broadcast(qb, qrow)

        # ================= preload tail weights (reuses wqp zone) ==========
        wtail = ctx.enter_context(tc.tile_pool(name="wtail", bufs=1))
        UpT = load_w_T("U_p", D_S, D_P, wtail, "UpT")      # [128, 2, 512]
        UwT = load_w_T("U_w", D_S, D_W, wtail, "UwT")
        AUT = load_w_T("A_U", D_W, D_W // 4, wtail, "AUT")  # [64, 1, 256]
        wproj = [load_w(f"Wproj{l}", DIMS[l], D_C, wtail) for l in range(NLVL)]
        wmix = load_w("Wmix", NLVL * D_C, NLVL, wtail)
        wz = load_w("Wz", D_E + D_C, D_P, wtail)
        wg = load_w("Wg", D_S + D_E + D_C, D_S, wtail)
        wav = load_w("A_V", D_W, D_W // 4, wtail)
        wmod = load_w("Wmod", D_C, D_W, wtail)
        wB = load_w("WB", D_C + D_P, D_W, wtail)
        wp1 = load_w("Wp1", D_P + D_C, H2, wtail)
        wp2 = load_w("Wp2", H2, D_P, wtail)
        # Wwg: first 1024 rows in rhs layout, last row separately
        wwg = []
        wwg_last = []
        for l in range(NLVL):
            t = pt(wtail, (128, 8, 1), f"wwg{l}")
            src = h[f"Wwg{l}"][:][0:1024, :].rearrange("(c p) o -> p c o", p=128)
            nc.sync.dma_start(out=t, in_=src)
            wwg.append(t)
            tl = st((1, 1), f"wwgl{l}")
            nc.sync.dma_start(out=tl, in_=h[f"Wwg{l}"][:][1024:1025, :])
            wwg_last.append(tl)
        # CMSWrite value/key weights are loaded lazily at the tail
        wlazy = ctx.enter_context(tc.tile_pool(name="wlazy", bufs=2))

        # ================= streaming: attention read + decay write =========
        scores = []
        allsums = []
        md0 = []
        kd0 = []
        ctx_cat = st((1, NLVL * D_C), "ctx_cat")
        craw = st((1, 256), "craw")
        for l in range(NLVL):
            N = SIZES[l]
            keep = 1.0 - DECAYS[l]
            T = N // SUB
            sc = st((128, T), f"sc{l}")
            scores.append(sc)
            mview = h[f"M{l}"][:].rearrange("(s c p) d -> s p c d", c=SUP_M, p=128)
            moview = h[f"Mo{l}"][:].rearrange("(s c p) d -> s p c d", c=SUP_M, p=128)
            kview = h[f"K{l}"][:].rearrange("(s c p) d -> s p c d", c=SUP_K, p=128)
            koview = h[f"Ko{l}"][:].rearrange("(s c p) d -> s p c d", c=SUP_K, p=128)

            # ---- K pass: scores + decay + writeout ----
            for s in range(N // (SUB * SUP_K)):
                ksb = pt(kpool, (128, SUP_K, D_K), "ksb")
                nc.sync.dma_start(out=ksb, in_=kview[s])
                prod = pt(prodp, (128, SUP_K, D_K), "prod")
                nc.vector.tensor_mul(prod, ksb, bcast_mid(qb, SUP_K))
                nc.vector.reduce_sum(sc[:, s * SUP_K:(s + 1) * SUP_K], prod, axis=AX.X)
                nc.scalar.mul(ksb, ksb, keep)
                if s == 0:
                    kd = st((1, D_K), f"kd0_{l}")
                    nc.gpsimd.tensor_copy(kd, ksb[0:1, 0, :])
                    kd0.append(kd)
                    # exclude row 0 from the bulk writeout (patched later)
                    nc.sync.dma_start(out=koview[0][1:, 0, :], in_=ksb[1:, 0, :])
                    nc.sync.dma_start(out=koview[0][:, 1:, :], in_=ksb[:, 1:, :])
                else:
                    nc.sync.dma_start(out=koview[s], in_=ksb)

            # ---- softmax stats ----
            rowmax = st((128, 1), f"rmax{l}")
            nc.vector.reduce_max(rowmax, sc, axis=AX.X)
            allmax = st((128, 1), f"amax{l}")
            nc.gpsimd.partition_all_reduce(allmax, rowmax, channels=128,
                                           reduce_op=bass_isa.ReduceOp.max)
            negb = st((128, 1), f"negb{l}")
            nc.vector.tensor_scalar_mul(negb, allmax, -SCALE)
            rowsum = st((128, 1), f"rsum{l}")
            nc.scalar.activation(sc, sc, AF.Exp, bias=negb, scale=SCALE,
                                 accum_out=rowsum)
            allsum = st((128, 1), f"asum{l}")
            nc.gpsimd.partition_all_reduce(allsum, rowsum, channels=128,
                                           reduce_op=bass_isa.ReduceOp.add)
            allsums.append(allsum)

            # ---- M pass: context matmul + decay + writeout ----
            pc = pt(pctx, (1, DIMS[l]), "pctxt")
            for s in range(N // (SUB * SUP_M)):
                msb = pt(mpool, (128, SUP_M, DIMS[l]), "msb")
                nc.sync.dma_start(out=msb, in_=mview[s])
                for c in range(SUP_M):
                    g = s * SUP_M + c
                    nc.tensor.matmul(pc, lhsT=sc[:, g:g + 1], rhs=msb[:, c, :],
                                     start=(g == 0), stop=(g == T - 1))
                nc.vector.tensor_scalar_mul(msb, msb, keep)
                if s == 0:
                    md = st((1, DIMS[l]), f"md0_{l}")
                    nc.gpsimd.tensor_copy(md, msb[0:1, 0, :])
                    md0.append(md)
                    nc.sync.dma_start(out=moview[0][1:, 0, :], in_=msb[1:, 0, :])
                    nc.sync.dma_start(out=moview[0][:, 1:, :], in_=msb[:, 1:, :])
                else:
                    nc.sync.dma_start(out=moview[s], in_=msb)

            # c_l = pc / allsum ; then ctx_l = c_l @ Wproj_l
            inv = st((1, 1), f"inv{l}")
            nc.vector.reciprocal(inv, allsum[0:1, :])
            nc.vector.tensor_scalar_mul(craw, pc, inv)
            ccols_l = bounce(craw, DIMS[l], f"cr{l}")
            mv(ccols_l, wproj[l], D_C, ctx_cat[:, l * D_C:(l + 1) * D_C])

        # ================= mixing =================
        mixcols = bounce(ctx_cat, NLVL * D_C, "mixc")
        mixrow = st((1, NLVL), "mixrow")
        mv(mixcols, wmix, NLVL, mixrow, bias_row=rows["bmix"])
        mmax = st((1, 1), "mmax")
        nc.vector.reduce_max(mmax, mixrow, axis=AX.X)
        mneg = st((1, 1), "mneg")
        nc.vector.tensor_scalar_mul(mneg, mmax, -1.0)
        msum = st((1, 1), "msum")
        nc.scalar.activation(mixrow, mixrow, AF.Exp, bias=mneg, accum_out=msum)
        minv = st((1, 1), "minv")
        nc.vector.reciprocal(minv, msum)
        nc.vector.tensor_scalar_mul(mixrow, mixrow, minv)

        c_t = st((1, D_C), "c_t")
        tmp_c = st((1, D_C), "tmp_c")
        nc.vector.tensor_scalar_mul(c_t, ctx_cat[:, 0:D_C], mixrow[:, 0:1])
        for l in range(1, NLVL):
            nc.vector.tensor_scalar_mul(tmp_c, ctx_cat[:, l * D_C:(l + 1) * D_C],
                                        mixrow[:, l:l + 1])
            nc.vector.tensor_add(c_t, c_t, tmp_c)

        # ================= HOPE core =================
        ccols = bounce(c_t, D_C, "ccols")          # [128, 2]
        ecols = st((128, 4), "ecols")
        nc.vector.tensor_copy(ecols, x0c[:, 4:8])
        scols = st((128, 4), "scols")
        nc.vector.tensor_copy(scols, x0c[:, 0:4])
        pcols = cols_from_dram(h["p_prev"][0, :], D_P, "pcols")
        wcols = cols_from_dram(h["w_prev"][0, :], D_W, "wcols")

        # z_t = [e, c] @ Wz + bz
        zin = st((128, 6), "zin")
        nc.vector.tensor_copy(zin[:, 0:4], ecols)
        nc.vector.tensor_copy(zin[:, 4:6], ccols)
        z_t = st((1, D_P), "z_t")
        mv(zin, wz, D_P, z_t, bias_row=rows["bz"])

        # g_t = sigmoid([s, e, c] @ Wg + bg)
        gin = st((128, 10), "gin")
        nc.vector.tensor_copy(gin[:, 0:4], scols)
        nc.vector.tensor_copy(gin[:, 4:8], ecols)
        nc.vector.tensor_copy(gin[:, 8:10], ccols)
        g_t = st((1, D_S), "g_t")
        mv(gin, wg, D_S, g_t, bias_row=rows["bg"], act=AF.Sigmoid)

        # wave: w_t = (tanh(A_diag)*0.9 * w_prev + A_U @ (A_V.T @ w_prev)) * modu + Bv
        drow = st((1, D_W), "drow")
        nc.scalar.activation(drow, rows["A_diag"], AF.Tanh)
        nc.vector.tensor_scalar_mul(drow, drow, 0.9)
        diag_term = st((1, D_W), "diag_term")
        nc.vector.tensor_mul(diag_term, drow, rows["w_prev"])
        t1 = st((1, D_W // 4), "t1")
        mv(wcols, wav, D_W // 4, t1)
        t1c = bounce(t1, D_W // 4, "t1c")          # [64, 1]
        t2 = st((1, D_W), "t2")
        mv(t1c, AUT, D_W, t2, nK=1)
        aw = st((1, D_W), "aw")
        nc.vector.tensor_add(aw, diag_term, t2)
        modu = st((1, D_W), "modu")
        mv(ccols, wmod, D_W, modu, bias_row=rows["bmod"], act=AF.Tanh)
        zcols = bounce(z_t, D_P, "zcols")
        bin_ = st((128, 4), "bin")
        nc.vector.tensor_copy(bin_[:, 0:2], ccols)
        nc.vector.tensor_copy(bin_[:, 2:4], zcols)
        bv = st((1, D_W), "bv")
        mv(bin_, wB, D_W, bv, bias_row=rows["bB"])
        w_t = st((1, D_W), "w_t")
        nc.vector.tensor_mul(w_t, aw, modu)
        nc.vector.tensor_add(w_t, w_t, bv)

        # particle: p_t = p_prev + Wp2 @ relu(ln(Wp1 @ [p, c]))
        pin = st((128, 4), "pin")
        nc.vector.tensor_copy(pin[:, 0:2], pcols)
        nc.vector.tensor_copy(pin[:, 2:4], ccols)
        hrow = st((1, H2), "hrow")
        mv(pin, wp1, H2, hrow, bias_row=rows["bp1"])
        layer_norm_relu(hrow, H2, rows["gp"], rows["betp"], "lnp")
        hcols = bounce(hrow, H2, "hcols")
        p_t = st((1, D_P), "p_t")
        mv(hcols, wp2, D_P, p_t, bias_row=rows["bp2"])
        nc.vector.tensor_add(p_t, p_t, rows["p_prev"])

        # s_t = s_prev + g * (p_t @ U_p.T) + (1 - g) * (w_t @ U_w.T)
        ptc = bounce(p_t, D_P, "ptc")
        wtc = bounce(w_t, D_W, "wtc")
        yp = st((1, D_S), "yp")
        mv(ptc, UpT, D_S, yp)
        yw = st((1, D_S), "yw")
        mv(wtc, UwT, D_S, yw)
        omg = st((1, D_S), "omg")
        nc.scalar.activation(omg, g_t, AF.Copy, bias=1.0, scale=-1.0)
        s_t = st((1, D_S), "s_trow")
        nc.vector.tensor_mul(yp, yp, g_t)
        nc.vector.tensor_mul(yw, yw, omg)
        nc.vector.tensor_add(s_t, yp, yw)
        nc.vector.tensor_add(s_t, s_t, rows["s_prev"])
        nc.sync.dma_start(out=h["s_t"][:], in_=s_t)

        # ================= CMSWrite row-0 patches =================
        stc = bounce(s_t, D_S, "stc")
        wic = st((128, 8), "wic")
        nc.vector.tensor_copy(wic[:, 0:4], stc)
        nc.vector.tensor_copy(wic[:, 4:8], ecols)
        for l in range(NLVL):
            gw = st((1, 1), f"gw{l}")
            mv(wic, wwg[l], 1, gw)
            extra = st((1, 1), f"gex{l}")
            nc.vector.tensor_mul(extra, rows["r_t"], wwg_last[l])
            nc.vector.tensor_add(gw, gw, extra)
            nc.vector.tensor_add(gw, gw, rows[f"bwg{l}"])
            nc.scalar.activation(gw, gw, AF.Sigmoid)
            omgw = st((1, 1), f"omgw{l}")
            nc.scalar.activation(omgw, gw, AF.Copy, bias=1.0, scale=-1.0)

            wwv_l = load_w(f"Wwv{l}", D_S + D_E, DIMS[l], wlazy, tag="lzv")
            vw = st((1, DIMS[l]), f"vw{l}")
            mv(wic, wwv_l, DIMS[l], vw, bias_row=rows[f"bwv{l}"])
            wwk_l = load_w(f"Wwk{l}", D_S + D_E, D_K, wlazy, tag="lzk")
            kw = st((1, D_K), f"kw{l}")
            mv(wic, wwk_l, D_K, kw, bias_row=rows[f"bwk{l}"])

            newm = st((1, DIMS[l]), f"newm{l}")
            nc.vector.tensor_scalar_mul(newm, vw, gw)
            tmpm = st((1, DIMS[l]), f"tmpm{l}")
            nc.vector.tensor_scalar_mul(tmpm, md0[l], omgw)
            nc.vector.tensor_add(newm, newm, tmpm)
            nc.sync.dma_start(out=h[f"Mo{l}"][:][0:1, :], in_=newm)

            newk = st((1, D_K), f"newk{l}")
            nc.vector.tensor_scalar_mul(newk, kw, gw)
            tmpk = st((1, D_K), f"tmpk{l}")
            nc.vector.tensor_scalar_mul(tmpk, kd0[l], omgw)
            nc.vector.tensor_add(newk, newk, tmpk)
            nc.sync.dma_start(out=h[f"Ko{l}"][:][0:1, :], in_=newk)

    nc.finalize()
    return nc


_CACHE = {}


def _get_nc():
    if "nc" not in _CACHE:
        _CACHE["nc"] = build_nc()
    return _CACHE["nc"]


def _run(inputs, trace=False):
    nc = _get_nc()
    f = lambda a: np.ascontiguousarray(np.asarray(a), dtype=np.float32)
    p = inputs["params"]
    shared = {
        "Wq1": f(p["Wq1"]), "bq1": f(p["bq1"])[None, :], "gq": f(p["gq"])[None, :],
        "betq": f(p["betq"])[None, :], "Wq2": f(p["Wq2"]), "bq2": f(p["bq2"])[None, :],
        "Wmix": f(p["Wmix"]), "bmix": f(p["bmix"])[None, :],
        "Wz": f(p["Wz"]), "bz": f(p["bz"])[None, :],
        "Wg": f(p["Wg"]), "bg": f(p["bg"])[None, :],
        "U_p": f(p["U_p"]), "U_w": f(p["U_w"]),
        "A_diag": f(p["A_diag"])[None, :], "A_U": f(p["A_U"]), "A_V": f(p["A_V"]),
        "Wmod": f(p["Wmod"]), "bmod": f(p["bmod"])[None, :],
        "WB": f(p["WB"]), "bB": f(p["bB"])[None, :],
        "Wp1": f(p["Wp1"]), "bp1": f(p["bp1"])[None, :], "gp": f(p["gp"])[None, :],
        "betp": f(p["betp"])[None, :],
        "Wp2": f(p["Wp2"]), "bp2": f(p["bp2"])[None, :],
    }
    for l in range(NLVL):
        shared[f"Wproj{l}"] = f(p["Wproj"][l])
        shared[f"Wwg{l}"] = f(p["Wwg"][l])
        shared[f"bwg{l}"] = f(p["bwg"][l])[None, :]
        shared[f"Wwv{l}"] = f(p["Wwv"][l])
        shared[f"bwv{l}"] = f(p["bwv"][l])[None, :]
        shared[f"Wwk{l}"] = f(p["Wwk"][l])
        shared[f"bwk{l}"] = f(p["bwk"][l])[None, :]

    full = {k: f(inputs[k]) for k in
            ["M0", "M1", "M2", "K0", "K1", "K2",
             "s_prev", "w_prev", "p_prev", "e_t", "r_t"]}
    in_maps = []
    for b in range(B):
        m = dict(shared)
        for l in range(NLVL):
            m[f"M{l}"] = full[f"M{l}"][b]
            m[f"K{l}"] = full[f"K{l}"][b]
        for k in ("s_prev", "w_prev", "p_prev", "e_t", "r_t"):
            m[k] = full[k][b][None, :]
        in_maps.append(m)

    from concourse.bass_utils import run_bass_kernel_spmd

    bkr = run_bass_kernel_spmd(nc, in_maps, list(range(B)), trace=trace)
    res = bkr.results
    s_t = np.stack([res[b]["s_t"][0] for b in range(B)])
    outs = [s_t]
    for pref in ("Mo", "Ko"):
        for l in range(NLVL):
            outs.append(np.stack([res[b][f"{pref}{l}"] for b in range(B)]))
    return tuple(outs), bkr


def kernel(M0, M1, M2, K0, K1, K2, s_prev, w_prev, p_prev, e_t, r_t, params):
    outs, _ = _run(dict(M0=M0, M1=M1, M2=M2, K0=K0, K1=K1, K2=K2,
                        s_prev=s_prev, w_prev=w_prev, p_prev=p_prev,
                        e_t=e_t, r_t=r_t, params=params))
    return outs


# revision 12
# speedup vs baseline: 1.0822x; 1.0822x over previous
"""Trainium2 Bass kernel for the CMS/HOPE memory cell.

Sharding: data-parallel over batch B=8 across 8 NeuronCores (one batch
element per core, no collectives). Per core:
  - q-network (small MLP) on TensorE
  - per level l: stream K_l once (scores on VectorE, decay-scale on ScalarE,
    write K_l*keep out), softmax stats, stream M_l once (context via f32r
    PSUM matmul accumulation, fp32 decay-scale on VectorE, write M_l*keep),
  - mixing + HOPE core + CMSWrite row-0 patches on TensorE/VectorE/ScalarE.
Streams use partition-major layout (8KB contiguous per partition per DMA).
All shapes are hardcoded; kernel() takes full unsharded inputs.
"""

import sys

sys.path.insert(0, "/opt/trn_rl_repo")

import math
from contextlib import ExitStack

import numpy as np

import concourse.bass as bass
import concourse.tile as tile
from concourse import bacc
from concourse import bass_isa, mybir
from concourse.masks import make_identity

F32 = mybir.dt.float32
F32R = mybir.dt.float32r
AF = mybir.ActivationFunctionType
AX = mybir.AxisListType

B = 8
D_S, D_E, D_K, D_C, D_P, D_W = 512, 512, 128, 256, 256, 256
SIZES = [32768, 8192, 2048]
DIMS = [256, 256, 256]
DECAYS = [0.001, 0.01, 0.1]
NLVL = 3
SCALE = 1.0 / math.sqrt(float(D_K))
SUP_K = 16         # rows per partition per K super-tile (2048 rows, 1 MiB)
SUP_M = 8          # rows per partition per M super-tile (1024 rows, 1 MiB)
H1 = D_S + D_E     # 1024
H2 = D_P + D_C     # 512


def build_nc():
    nc = bacc.Bacc()

    # ---------------- I/O declarations (per-core slices) ----------------
    h = {}

    def din(name, shape):
        h[name] = nc.declare_dram_parameter(name, list(shape), F32, isOutput=False)
        return h[name]

    def dout(name, shape):
        h[name] = nc.declare_dram_parameter(name, list(shape), F32, isOutput=True)
        return h[name]

    for l in range(NLVL):
        din(f"M{l}", (SIZES[l], DIMS[l]))
        din(f"K{l}", (SIZES[l], D_K))
    din("s_prev", (1, D_S))
    din("w_prev", (1, D_W))
    din("p_prev", (1, D_P))
    din("e_t", (1, D_E))
    din("r_t", (1, 1))
    # params
    din("Wq1", (H1, H1)); din("bq1", (1, H1)); din("gq", (1, H1)); din("betq", (1, H1))
    din("Wq2", (H1, D_K)); din("bq2", (1, D_K))
    for l in range(NLVL):
        din(f"Wproj{l}", (DIMS[l], D_C))
    din("Wmix", (NLVL * D_C, NLVL)); din("bmix", (1, NLVL))
    din("Wz", (D_E + D_C, D_P)); din("bz", (1, D_P))
    din("Wg", (D_S + D_E + D_C, D_S)); din("bg", (1, D_S))
    din("U_p", (D_S, D_P)); din("U_w", (D_S, D_W))
    din("A_diag", (1, D_W)); din("A_U", (D_W, D_W // 4)); din("A_V", (D_W, D_W // 4))
    din("Wmod", (D_C, D_W)); din("bmod", (1, D_W))
    din("WB", (D_C + D_P, D_W)); din("bB", (1, D_W))
    din("Wp1", (D_P + D_C, H2)); din("bp1", (1, H2)); din("gp", (1, H2)); din("betp", (1, H2))
    din("Wp2", (H2, D_P)); din("bp2", (1, D_P))
    for l in range(NLVL):
        din(f"Wwg{l}", (D_S + D_E + 1, 1)); din(f"bwg{l}", (1, 1))
        din(f"Wwv{l}", (D_S + D_E, DIMS[l])); din(f"bwv{l}", (1, DIMS[l]))
        din(f"Wwk{l}", (D_S + D_E, D_K)); din(f"bwk{l}", (1, D_K))

    dout("s_t", (1, D_S))
    for l in range(NLVL):
        dout(f"Mo{l}", (SIZES[l], DIMS[l]))
        dout(f"Ko{l}", (SIZES[l], D_K))

    with tile.TileContext(nc) as tc, ExitStack() as ctx:
        spool = ctx.enter_context(tc.tile_pool(name="spool", bufs=1))
        wchunk = ctx.enter_context(tc.tile_pool(name="wchunk", bufs=2))
        mpool = ctx.enter_context(tc.tile_pool(name="mpool", bufs=3))
        mrpool = ctx.enter_context(tc.tile_pool(name="mrpool", bufs=2))
        kpool = ctx.enter_context(tc.tile_pool(name="kpool", bufs=2))
        prodp = ctx.enter_context(tc.tile_pool(name="prodp", bufs=1))
        pctx = ctx.enter_context(tc.tile_pool(name="pctx", bufs=2, space="PSUM"))
        psmall = ctx.enter_context(tc.tile_pool(name="psmall", bufs=2, space="PSUM"))
        ptr = ctx.enter_context(tc.tile_pool(name="ptr", bufs=2, space="PSUM"))

        def pt(pool, shape, tag, dtype=F32):
            return pool.tile(list(shape), dtype, name=tag, tag=tag)

        def st(shape, tag, dtype=F32):
            return pt(spool, shape, tag, dtype)

        def load_row(name, D, pool=None):
            t = pt(pool or spool, (1, D), "r_" + name)
            nc.sync.dma_start(out=t, in_=h[name][:])
            return t

        ident1 = st((1, 1), "ident1")
        nc.vector.memset(ident1, 1.0)
        ident = st((128, 128), "ident")
        make_identity(nc, ident)
        eps_t = st((1, 1), "eps_t")
        nc.vector.memset(eps_t, 1e-6)

        rows = {}
        for name, D in [
            ("s_prev", D_S), ("w_prev", D_W), ("p_prev", D_P), ("e_t", D_E),
            ("r_t", 1), ("bq2", D_K),
            ("bmix", NLVL), ("bz", D_P), ("bg", D_S), ("A_diag", D_W),
            ("bmod", D_W), ("bB", D_W), ("bp1", H2), ("gp", H2), ("betp", H2),
            ("bp2", D_P),
        ]:
            rows[name] = load_row(name, D)
        for l in range(NLVL):
            for name, D in [(f"bwg{l}", 1), (f"bwv{l}", DIMS[l]), (f"bwk{l}", D_K)]:
                rows[name] = load_row(name, D)

        # ---- helpers ----
        def cols_from_dram(ap_1d, D, tag, dest=None, dcol0=0):
            """DRAM [D] -> SBUF cols tile [p, D/p] with cols[p, c] = x[c*p + p]."""
            p = min(D, 128)
            k = D // p
            t = dest if dest is not None else st((p, k), tag)
            src = ap_1d.rearrange("(c p) -> p c", p=p)
            if dest is None:
                nc.sync.dma_start(out=t, in_=src)
                return t
            nc.sync.dma_start(out=t[:, dcol0:dcol0 + k], in_=src)
            return t

        def redist(row_ap, D, tag, dest=None, dcol0=0):
            """SBUF row [1, D] -> SBUF cols [p, D/p] via PE transposes."""
            p = min(D, 128)
            k = D // p
            t = dest if dest is not None else st((p, k), tag)
            for c in range(k):
                tp = pt(ptr, (p, 1), "rd_ps")
                nc.tensor.transpose(tp, row_ap[:, c * p:(c + 1) * p], ident1)
                nc.vector.tensor_copy(t[:, dcol0 + c:dcol0 + c + 1], tp)
            return t

        def load_w(name, Din, Dout, pool, tag=None):
            """W [Din, Dout] -> SBUF [128, Din/128, Dout] (rhs layout)."""
            k = Din // 128
            t = pt(pool, (128, k, Dout), tag or ("w_" + name))
            nc.sync.dma_start(out=t, in_=h[name][:].rearrange("(c p) o -> p c o", p=128))
            return t

        def mv(xcols, Wsb, Dout, out_row, bias_row=None, act=None, nK=None):
            """out_row[1, Dout] = act(xcols-vector @ W + bias)."""
            k = nK if nK is not None else Wsb.shape[1]
            for f0 in range(0, Dout, 512):
                f1 = min(Dout, f0 + 512)
                ps = pt(psmall, (1, f1 - f0), "psmv")
                for c in range(k):
                    nc.tensor.matmul(ps, lhsT=xcols[:, c:c + 1], rhs=Wsb[:, c, f0:f1],
                                     start=(c == 0), stop=(c == k - 1))
                if bias_row is not None:
                    nc.vector.tensor_add(out_row[:, f0:f1], ps, bias_row[:, f0:f1])
                else:
                    nc.vector.tensor_copy(out_row[:, f0:f1], ps)
            if act is not None:
                nc.scalar.activation(out_row, out_row, act)

        def load_w_T(name, Din, Dout, pool, tag):
            """W [Din, Dout] -> transposed SBUF tile [pc, Dout/pc, Din] so that
            WT[p, jc, i] = W[i, jc*pc + p] (rhs layout for x @ W.T)."""
            pc = min(Dout, 128)
            cc = Dout // pc
            rc = Din // 128
            WT = pt(pool, (pc, cc, Din), tag)
            for r in range(rc):
                chk = pt(wchunk, (128, Dout), "wtchunk")
                nc.sync.dma_start(out=chk, in_=h[name][:][r * 128:(r + 1) * 128, :])
                for j in range(cc):
                    p_ = pt(ptr, (pc, 128), "ptrt")
                    nc.tensor.transpose(p_, chk[:, j * pc:(j + 1) * pc], ident)
                    nc.vector.tensor_copy(WT[:, j, r * 128:(r + 1) * 128], p_)
            return WT

        def layer_norm_relu(rw, Dn, g_row, b_row, tag):
            nsub = (Dn + 511) // 512
            stats = st((1, nsub, 6), tag + "_st")
            for i in range(nsub):
                nc.vector.bn_stats(stats[:, i, :], rw[:, i * 512:(i + 1) * 512])
            mvv = st((1, 2), tag + "_mv")
            nc.vector.bn_aggr(mvv, stats)
            sd = st((1, 1), tag + "_sd")
            nc.scalar.activation(sd, mvv[:, 1:2], AF.Sqrt, bias=eps_t)
            rs = st((1, 1), tag + "_rs")
            nc.vector.reciprocal(rs, sd)
            nc.vector.tensor_scalar_sub(rw, rw, mvv[:, 0:1])
            nc.vector.tensor_scalar_mul(rw, rw, rs)
            nc.vector.tensor_mul(rw, rw, g_row)
            nc.vector.tensor_add(rw, rw, b_row)
            nc.scalar.activation(rw, rw, AF.Relu)

        def bcast_mid(ap2d, n):
            return bass.AP(tensor=ap2d.tensor, offset=ap2d.offset,
                           ap=[ap2d.ap[0], [0, n], ap2d.ap[1]])

        # ================= q network (weights in a released pool) ==========
        x0c = st((128, 8), "x0c")
        cols_from_dram(h["s_prev"][0, :], D_S, None, dest=x0c, dcol0=0)
        cols_from_dram(h["e_t"][0, :], D_E, None, dest=x0c, dcol0=4)
        qb = st((128, D_K), "qb")
        qrow = st((1, D_K), "qrow")

        with tc.tile_pool(name="wqp", bufs=1) as wqp:
            wq1 = load_w("Wq1", H1, H1, wqp)
            wq2 = load_w("Wq2", H1, D_K, wqp)
            bq1r = load_row("bq1", H1, wqp)
            gqr = load_row("gq", H1, wqp)
            betqr = load_row("betq", H1, wqp)
            q1row = pt(wqp, (1, H1), "q1row")
            mv(x0c, wq1, H1, q1row, bias_row=bq1r)
            layer_norm_relu(q1row, H1, gqr, betqr, "lnq")
            qcols = redist(q1row, H1, "qc")
            mv(qcols, wq2, D_K, qrow, bias_row=rows["bq2"])
            nc.gpsimd.partition_broadcast(qb, qrow)

        # ================= preload tail weights (reuses wqp zone) ==========
        wtail = ctx.enter_context(tc.tile_pool(name="wtail", bufs=1))
        UpT = load_w_T("U_p", D_S, D_P, wtail, "UpT")      # [128, 2, 512]
        UwT = load_w_T("U_w", D_S, D_W, wtail, "UwT")
        AUT = load_w_T("A_U", D_W, D_W // 4, wtail, "AUT")  # [64, 1, 256]
        wproj = [load_w(f"Wproj{l}", DIMS[l], D_C, wtail) for l in range(NLVL)]
        wmix = load_w("Wmix", NLVL * D_C, NLVL, wtail)
        wz = load_w("Wz", D_E + D_C, D_P, wtail)
        wg = load_w("Wg", D_S + D_E + D_C, D_S, wtail)
        wav = load_w("A_V", D_W, D_W // 4, wtail)
        wmod = load_w("Wmod", D_C, D_W, wtail)
        wB = load_w("WB", D_C + D_P, D_W, wtail)
        wp1 = load_w("Wp1", D_P + D_C, H2, wtail)
        wp2 = load_w("Wp2", H2, D_P, wtail)
        wwg = []
        wwg_last = []
        for l in range(NLVL):
            t = pt(wtail, (128, 8, 1), f"wwg{l}")
            src = h[f"Wwg{l}"][:][0:1024, :].rearrange("(c p) o -> p c o", p=128)
            nc.sync.dma_start(out=t, in_=src)
            wwg.append(t)
            tl = st((1, 1), f"wwgl{l}")
            nc.sync.dma_start(out=tl, in_=h[f"Wwg{l}"][:][1024:1025, :])
            wwg_last.append(tl)
        # CMSWrite value/key weights are loaded lazily at the tail
        wlazy = ctx.enter_context(tc.tile_pool(name="wlazy", bufs=1))

        # ================= streaming: attention read + decay write =========
        md0 = []
        kd0 = []
        ctx_cat = st((1, NLVL * D_C), "ctx_cat")
        craw = st((1, 256), "craw")
        for l in range(NLVL):
            N = SIZES[l]
            keep = 1.0 - DECAYS[l]
            T = N // 128
            sc = st((128, T), f"sc{l}")              # raw scores (fp32)
            scr = st((128, T), f"scr{l}", F32R)      # exp'd attn weights (f32r)
            # partition-major: row n = s*(128*SUP) + p*SUP + c
            mview = h[f"M{l}"][:].rearrange("(s p c) d -> s p c d", c=SUP_M, p=128)
            moview = h[f"Mo{l}"][:].rearrange("(s p c) d -> s p c d", c=SUP_M, p=128)
            kview = h[f"K{l}"][:].rearrange("(s p c) d -> s p c d", c=SUP_K, p=128)
            koview = h[f"Ko{l}"][:].rearrange("(s p c) d -> s p c d", c=SUP_K, p=128)

            # ---- K pass: scores + decay + writeout ----
            for s in range(N // (128 * SUP_K)):
                ksb = pt(kpool, (128, SUP_K, D_K), "ksb")
                nc.sync.dma_start(out=ksb, in_=kview[s])
                for hs in range(2):
                    hk = SUP_K // 2
                    prod = pt(prodp, (128, hk, D_K), "prod")
                    nc.vector.tensor_mul(prod, ksb[:, hs * hk:(hs + 1) * hk, :],
                                         bcast_mid(qb, hk))
                    nc.vector.reduce_sum(sc[:, s * SUP_K + hs * hk:s * SUP_K + (hs + 1) * hk],
                                         prod, axis=AX.X)
                nc.scalar.mul(ksb, ksb, keep)
                if s == 0:
                    kd = st((1, D_K), f"kd0_{l}")
                    nc.gpsimd.tensor_copy(kd, ksb[0:1, 0, :])
                    kd0.append(kd)
                    # exclude row 0 (= partition 0, c 0) from the bulk writeout
                    nc.sync.dma_start(out=koview[0][0:1, 1:, :], in_=ksb[0:1, 1:, :])
                    nc.sync.dma_start(out=koview[0][1:, :, :], in_=ksb[1:, :, :])
                else:
                    nc.sync.dma_start(out=koview[s], in_=ksb)

            # ---- softmax stats ----
            rowmax = st((128, 1), f"rmax{l}")
            nc.vector.reduce_max(rowmax, sc, axis=AX.X)
            allmax = st((128, 1), f"amax{l}")
            nc.gpsimd.partition_all_reduce(allmax, rowmax, channels=128,
                                           reduce_op=bass_isa.ReduceOp.max)
            negb = st((128, 1), f"negb{l}")
            nc.vector.tensor_scalar_mul(negb, allmax, -SCALE)
            rowsum = st((128, 1), f"rsum{l}")
            nc.scalar.activation(scr, sc, AF.Exp, bias=negb, scale=SCALE,
                                 accum_out=rowsum)
            allsum = st((128, 1), f"asum{l}")
            nc.gpsimd.partition_all_reduce(allsum, rowsum, channels=128,
                                           reduce_op=bass_isa.ReduceOp.add)

            # ---- M pass: f32r context matmul + decay + writeout ----
            pc_ = pt(pctx, (1, DIMS[l]), "pctxt")
            for s in range(N // (128 * SUP_M)):
                msb = pt(mpool, (128, SUP_M, DIMS[l]), "msb")
                nc.sync.dma_start(out=msb, in_=mview[s])
                for hs in range(2):
                    hm = SUP_M // 2
                    msr = pt(mrpool, (128, hm, DIMS[l]), "msr", F32R)
                    nc.scalar.activation(msr, msb[:, hs * hm:(hs + 1) * hm, :],
                                         AF.Copy, scale=keep)
                    for c in range(hm):
                        g = s * SUP_M + hs * hm + c
                        nc.tensor.matmul(pc_, lhsT=scr[:, g:g + 1], rhs=msr[:, c, :],
                                         start=(g == 0), stop=(g == T - 1))
                nc.vector.tensor_scalar_mul(msb, msb, keep)
                if s == 0:
                    md = st((1, DIMS[l]), f"md0_{l}")
                    nc.gpsimd.tensor_copy(md, msb[0:1, 0, :])
                    md0.append(md)
                    nc.sync.dma_start(out=moview[0][0:1, 1:, :], in_=msb[0:1, 1:, :])
                    nc.sync.dma_start(out=moview[0][1:, :, :], in_=msb[1:, :, :])
                else:
                    nc.sync.dma_start(out=moview[s], in_=msb)

            # c_l = pc_ / (keep * allsum) ; ctx_l = c_l @ Wproj_l
            asml = st((1, 1), f"asml{l}")
            nc.vector.tensor_scalar_mul(asml, allsum[0:1, :], keep)
            inv = st((1, 1), f"inv{l}")
            nc.vector.reciprocal(inv, asml)
            nc.vector.tensor_scalar_mul(craw, pc_, inv)
            ccols_l = redist(craw, DIMS[l], f"cr{l}")
            mv(ccols_l, wproj[l], D_C, ctx_cat[:, l * D_C:(l + 1) * D_C])

        # ================= mixing =================
        mixcols = redist(ctx_cat, NLVL * D_C, "mixc")
        mixrow = st((1, NLVL), "mixrow")
        mv(mixcols, wmix, NLVL, mixrow, bias_row=rows["bmix"])
        mmax = st((1, 1), "mmax")
        nc.vector.reduce_max(mmax, mixrow, axis=AX.X)
        mneg = st((1, 1), "mneg")
        nc.vector.tensor_scalar_mul(mneg, mmax, -1.0)
        msum = st((1, 1), "msum")
        nc.scalar.activation(mixrow, mixrow, AF.Exp, bias=mneg, accum_out=msum)
        minv = st((1, 1), "minv")
        nc.vector.reciprocal(minv, msum)
        nc.vector.tensor_scalar_mul(mixrow, mixrow, minv)

        c_t = st((1, D_C), "c_t")
        tmp_c = st((1, D_C), "tmp_c")
        nc.vector.tensor_scalar_mul(c_t, ctx_cat[:, 0:D_C], mixrow[:, 0:1])
        for l in range(1, NLVL):
            nc.vector.tensor_scalar_mul(tmp_c, ctx_cat[:, l * D_C:(l + 1) * D_C],
                                        mixrow[:, l:l + 1])
            nc.vector.tensor_add(c_t, c_t, tmp_c)

        # ================= HOPE core =================
        ccols = redist(c_t, D_C, "ccols")          # [128, 2]
        ecols = st((128, 4), "ecols")
        nc.vector.tensor_copy(ecols, x0c[:, 4:8])
        scols = st((128, 4), "scols")
        nc.vector.tensor_copy(scols, x0c[:, 0:4])
        pcols = cols_from_dram(h["p_prev"][0, :], D_P, "pcols")
        wcols = cols_from_dram(h["w_prev"][0, :], D_W, "wcols")

        # z_t = [e, c] @ Wz + bz
        zin = st((128, 6), "zin")
        nc.vector.tensor_copy(zin[:, 0:4], ecols)
        nc.vector.tensor_copy(zin[:, 4:6], ccols)
        z_t = st((1, D_P), "z_t")
        mv(zin, wz, D_P, z_t, bias_row=rows["bz"])

        # g_t = sigmoid([s, e, c] @ Wg + bg)
        gin = st((128, 10), "gin")
        nc.vector.tensor_copy(gin[:, 0:4], scols)
        nc.vector.tensor_copy(gin[:, 4:8], ecols)
        nc.vector.tensor_copy(gin[:, 8:10], ccols)
        g_t = st((1, D_S), "g_t")
        mv(gin, wg, D_S, g_t, bias_row=rows["bg"], act=AF.Sigmoid)

        # wave: w_t = (tanh(A_diag)*0.9 * w_prev + A_U @ (A_V.T @ w_prev)) * modu + Bv
        drow = st((1, D_W), "drow")
        nc.scalar.activation(drow, rows["A_diag"], AF.Tanh)
        nc.vector.tensor_scalar_mul(drow, drow, 0.9)
        diag_term = st((1, D_W), "diag_term")
        nc.vector.tensor_mul(diag_term, drow, rows["w_prev"])
        t1 = st((1, D_W // 4), "t1")
        mv(wcols, wav, D_W // 4, t1)
        t1c = redist(t1, D_W // 4, "t1c")          # [64, 1]
        t2 = st((1, D_W), "t2")
        mv(t1c, AUT, D_W, t2, nK=1)
        aw = st((1, D_W), "aw")
        nc.vector.tensor_add(aw, diag_term, t2)
        modu = st((1, D_W), "modu")
        mv(ccols, wmod, D_W, modu, bias_row=rows["bmod"], act=AF.Tanh)
        zcols = redist(z_t, D_P, "zcols")
        bin_ = st((128, 4), "bin")
        nc.vector.tensor_copy(bin_[:, 0:2], ccols)
        nc.vector.tensor_copy(bin_[:, 2:4], zcols)
        bv = st((1, D_W), "bv")
        mv(bin_, wB, D_W, bv, bias_row=rows["bB"])
        w_t = st((1, D_W), "w_t")
        nc.vector.tensor_mul(w_t, aw, modu)
        nc.vector.tensor_add(w_t, w_t, bv)

        # particle: p_t = p_prev + Wp2 @ relu(ln(Wp1 @ [p, c]))
        pin = st((128, 4), "pin")
        nc.vector.tensor_copy(pin[:, 0:2], pcols)
        nc.vector.tensor_copy(pin[:, 2:4], ccols)
        hrow = st((1, H2), "hrow")
        mv(pin, wp1, H2, hrow, bias_row=rows["bp1"])
        layer_norm_relu(hrow, H2, rows["gp"], rows["betp"], "lnp")
        hcols = redist(hrow, H2, "hcols")
        p_t = st((1, D_P), "p_t")
        mv(hcols, wp2, D_P, p_t, bias_row=rows["bp2"])
        nc.vector.tensor_add(p_t, p_t, rows["p_prev"])

        # s_t = s_prev + g * (p_t @ U_p.T) + (1 - g) * (w_t @ U_w.T)
        ptc = redist(p_t, D_P, "ptc")
        wtc = redist(w_t, D_W, "wtc")
        yp = st((1, D_S), "yp")
        mv(ptc, UpT, D_S, yp)
        yw = st((1, D_S), "yw")
        mv(wtc, UwT, D_S, yw)
        omg = st((1, D_S), "omg")
        nc.scalar.activation(omg, g_t, AF.Copy, bias=1.0, scale=-1.0)
        s_t = st((1, D_S), "s_trow")
        nc.vector.tensor_mul(yp, yp, g_t)
        nc.vector.tensor_mul(yw, yw, omg)
        nc.vector.tensor_add(s_t, yp, yw)
        nc.vector.tensor_add(s_t, s_t, rows["s_prev"])
        nc.sync.dma_start(out=h["s_t"][:], in_=s_t)

        # ================= CMSWrite row-0 patches =================
        stc = redist(s_t, D_S, "stc")
        wic = st((128, 8), "wic")
        nc.vector.tensor_copy(wic[:, 0:4], stc)
        nc.vector.tensor_copy(wic[:, 4:8], ecols)
        for l in range(NLVL):
            gw = st((1, 1), f"gw{l}")
            mv(wic, wwg[l], 1, gw)
            extra = st((1, 1), f"gex{l}")
            nc.vector.tensor_mul(extra, rows["r_t"], wwg_last[l])
            nc.vector.tensor_add(gw, gw, extra)
            nc.vector.tensor_add(gw, gw, rows[f"bwg{l}"])
            nc.scalar.activation(gw, gw, AF.Sigmoid)
            omgw = st((1, 1), f"omgw{l}")
            nc.scalar.activation(omgw, gw, AF.Copy, bias=1.0, scale=-1.0)

            wwv_l = load_w(f"Wwv{l}", D_S + D_E, DIMS[l], wlazy, tag="lzv")
            vw = st((1, DIMS[l]), f"vw{l}")
            mv(wic, wwv_l, DIMS[l], vw, bias_row=rows[f"bwv{l}"])
            wwk_l = load_w(f"Wwk{l}", D_S + D_E, D_K, wlazy, tag="lzk")
            kw = st((1, D_K), f"kw{l}")
            mv(wic, wwk_l, D_K, kw, bias_row=rows[f"bwk{l}"])

            newm = st((1, DIMS[l]), f"newm{l}")
            nc.vector.tensor_scalar_mul(newm, vw, gw)
            tmpm = st((1, DIMS[l]), f"tmpm{l}")
            nc.vector.tensor_scalar_mul(tmpm, md0[l], omgw)
            nc.vector.tensor_add(newm, newm, tmpm)
            nc.sync.dma_start(out=h[f"Mo{l}"][:][0:1, :], in_=newm)

            newk = st((1, D_K), f"newk{l}")
            nc.vector.tensor_scalar_mul(newk, kw, gw)
            tmpk = st((1, D_K), f"tmpk{l}")
            nc.vector.tensor_scalar_mul(tmpk, kd0[l], omgw)
            nc.vector.tensor_add(newk, newk, tmpk)
            nc.sync.dma_start(out=h[f"Ko{l}"][:][0:1, :], in_=newk)

    nc.finalize()
    return nc


_CACHE = {}


def _get_nc():
    if "nc" not in _CACHE:
        _CACHE["nc"] = build_nc()
    return _CACHE["nc"]


def _run(inputs, trace=False):
    nc = _get_nc()
    f = lambda a: np.ascontiguousarray(np.asarray(a), dtype=np.float32)
    p = inputs["params"]
    shared = {
        "Wq1": f(p["Wq1"]), "bq1": f(p["bq1"])[None, :], "gq": f(p["gq"])[None, :],
        "betq": f(p["betq"])[None, :], "Wq2": f(p["Wq2"]), "bq2": f(p["bq2"])[None, :],
        "Wmix": f(p["Wmix"]), "bmix": f(p["bmix"])[None, :],
        "Wz": f(p["Wz"]), "bz": f(p["bz"])[None, :],
        "Wg": f(p["Wg"]), "bg": f(p["bg"])[None, :],
        "U_p": f(p["U_p"]), "U_w": f(p["U_w"]),
        "A_diag": f(p["A_diag"])[None, :], "A_U": f(p["A_U"]), "A_V": f(p["A_V"]),
        "Wmod": f(p["Wmod"]), "bmod": f(p["bmod"])[None, :],
        "WB": f(p["WB"]), "bB": f(p["bB"])[None, :],
        "Wp1": f(p["Wp1"]), "bp1": f(p["bp1"])[None, :], "gp": f(p["gp"])[None, :],
        "betp": f(p["betp"])[None, :],
        "Wp2": f(p["Wp2"]), "bp2": f(p["bp2"])[None, :],
    }
    for l in range(NLVL):
        shared[f"Wproj{l}"] = f(p["Wproj"][l])
        shared[f"Wwg{l}"] = f(p["Wwg"][l])
        shared[f"bwg{l}"] = f(p["bwg"][l])[None, :]
        shared[f"Wwv{l}"] = f(p["Wwv"][l])
        shared[f"bwv{l}"] = f(p["bwv"][l])[None, :]
        shared[f"Wwk{l}"] = f(p["Wwk"][l])
        shared[f"bwk{l}"] = f(p["bwk"][l])[None, :]

    full = {k: f(inputs[k]) for k in
            ["M0", "M1", "M2", "K0", "K1", "K2",
             "s_prev", "w_prev", "p_prev", "e_t", "r_t"]}
    in_maps = []
    for b in range(B):
        m = dict(shared)
        for l in range(NLVL):
            m[f"M{l}"] = full[f"M{l}"][b]
            m[f"K{l}"] = full[f"K{l}"][b]
        for k in ("s_prev", "w_prev", "p_prev", "e_t", "r_t"):
            m[k] = full[k][b][None, :]
        in_maps.append(m)

    from concourse.bass_utils import run_bass_kernel_spmd

    bkr = run_bass_kernel_spmd(nc, in_maps, list(range(B)), trace=trace)
    res = bkr.results
    s_t = np.stack([res[b]["s_t"][0] for b in range(B)])
    outs = [s_t]
    for pref in ("Mo", "Ko"):
        for l in range(NLVL):
            outs.append(np.stack([res[b][f"{pref}{l}"] for b in range(B)]))
    return tuple(outs), bkr


def kernel(M0, M1, M2, K0, K1, K2, s_prev, w_prev, p_prev, e_t, r_t, params):
    outs, _ = _run(dict(M0=M0, M1=M1, M2=M2, K0=K0, K1=K1, K2=K2,
                        s_prev=s_prev, w_prev=w_prev, p_prev=p_prev,
                        e_t=e_t, r_t=r_t, params=params))
    return outs
